# revision 22
# baseline (speedup 1.0000x reference)
"""Multi-head causal attention (B=2, S=2048, D=1024, H=16) on 8 trn2 cores.

Sharding: core c -> batch b=c//4, head-group g=c%4 (heads 4g..4g+3).

Host<->device traffic is minimized (it dominates the graded time):
- x ships d-sharded: core (b,g) gets xT rows [256g:256g+256) of its batch
  (1MB bf16); an on-chip AllGather over each batch quad rebuilds xT.
- weights ship deduplicated: cores c and c+4 hold the same head-slice, so
  each sends half (packed d-chunk-major into one [128,4096] buffer) and an
  AllGather over {c, c+4} rebuilds the full slice.
- out-projection partials are ReduceScattered on-chip per q-tile (bf16) and
  each core ships only its [4x128, 1024] bf16 share of the final output.
Total I/O ~24MB vs ~115MB for the naive full-duplication layout.

Compute schedule (per core: Q/K/V projections for its 4 heads, causal
attention in transposed layout, row-parallel out-projection partial):
- V projection in its own PSUM scope; pair-1 Q/K proj chunks interleaved
  into pair-0 attention to keep the PE warm under the ACT-bound exp stream.
- softmax normalize via StreamTranspose: the l-row is transposed so the
  DVE's iterative divide runs over 16 strided elements instead of 512.
- second causal straddle narrows the d=3 block to its live 128 columns.
- bias is added during PSUM evacuation (DVE) by the quad-rank-0 core only.
"""

import numpy as np

import concourse.bass as bass
import concourse.tile as tile
import concourse.mybir as mybir
from concourse import bacc
from concourse.bass_utils import run_bass_kernel_spmd

B, S, D, H, DH = 2, 2048, 1024, 16, 64
NCORES = 8
HPC = 4          # heads per core
PAIRS = 2        # head pairs per core
QT = 512         # q tile (free dim of scoresT / PV matmuls)
KB = 128         # k block (partition dim of scoresT)
NQT = S // QT    # 4
NKB = S // KB    # 16
DC = D // 128    # 8 contraction chunks for projections
XSH = D // 4     # 256 rows of xT shipped per core
SCALE = 1.0 / np.sqrt(DH)

QUADS = [[0, 1, 2, 3], [4, 5, 6, 7]]
PAIRS_RG = [[0, 4], [1, 5], [2, 6], [3, 7]]

F32 = mybir.dt.float32
BF = mybir.dt.bfloat16

ABLATE = set()
VARIANTS = set()


def _build(reps=None):
    import contextlib
    nc = bacc.Bacc("TRN2", target_bir_lowering=False, debug=False, num_devices=NCORES)

    xs = nc.dram_tensor("xs", [XSH, S], BF, kind="ExternalInput").ap()
    w_in = nc.dram_tensor("w_in", [128, 4096], BF, kind="ExternalInput").ap()
    bo_r = nc.dram_tensor("bo_r", [1, D], F32, kind="ExternalInput").ap()
    tri = nc.dram_tensor("tri", [KB, KB], BF, kind="ExternalInput").ap()
    # out rows j*128..j*128+128 = this core's RS share of q rows 512j+128g..
    out = nc.dram_tensor("out", [NQT * 128, D], BF, kind="ExternalOutput").ap()

    with tile.TileContext(nc) as tc, \
         (tc.For_i(0, reps, 1) if reps else contextlib.nullcontext()), \
         tc.tile_pool(name="persist", bufs=1) as persist, \
         tc.tile_pool(name="dram", bufs=1, space="DRAM") as dram:
        # ---- collective bounces ----
        xg_in = dram.tile([XSH, S], BF, name="xg_in")
        xg = dram.tile([D, S], BF, name="xg")
        w_ag_in = dram.tile([128, 4096], BF, name="w_ag_in")
        w_all = dram.tile([256, 4096], BF, name="w_all")
        rs_in = [dram.tile([512, D], BF, name=f"rs_in{j}") for j in range(NQT)]
        rs_out = [dram.tile([128, D], BF, name=f"rs_out{j}") for j in range(NQT)]

        # weights: pair AllGather (1MB, fires first), then x: quad AllGather
        nc.gpsimd.dma_start(w_ag_in[:], w_in[:])
        nc.gpsimd.collective_compute(
            "AllGather", mybir.AluOpType.bypass, replica_groups=PAIRS_RG,
            ins=[w_ag_in.opt()], outs=[w_all.opt()])
        nc.gpsimd.dma_start(xg_in[:], xs[:])
        nc.gpsimd.collective_compute(
            "AllGather", mybir.AluOpType.bypass, replica_groups=QUADS,
            ins=[xg_in.opt()], outs=[xg.opt()])

        # ---- persistent tiles ----
        qt_sb = [persist.tile([128, S], BF, name=f"qt{p}", tag=f"qt{p}") for p in range(PAIRS)]
        kt_sb = [persist.tile([128, S], BF, name=f"kt{p}", tag=f"kt{p}") for p in range(PAIRS)]
        vt_sb = [persist.tile([128, HPC * (DH + 1)], BF, name=f"vt{j}", tag=f"vt{j}") for j in range(NKB)]
        ctx_sb = [persist.tile([128, S], BF, name=f"ctx{p}", tag=f"ctx{p}") for p in range(PAIRS)]
        wo_sb = [persist.tile([128, D], BF, name=f"wo{p}", tag=f"wo{p}") for p in range(PAIRS)]
        tri_sb = persist.tile([KB, KB], BF, name="tri", tag="tri")
        bo_sb = persist.tile([1, D], F32, name="bo", tag="bo")
        bo_bc = persist.tile([128, D], F32, name="bo_bc", tag="bo_bc")

        xts = [persist.tile([128, S], BF, name=f"xts{i}", tag=f"xts{i}") for i in range(DC)]
        wq_sb = [persist.tile([128, HPC * DH], BF, name=f"wq{i}", tag=f"wq{i}") for i in range(DC)]
        wk_sb = [persist.tile([128, HPC * DH], BF, name=f"wk{i}", tag=f"wk{i}") for i in range(DC)]
        wv_sb = [persist.tile([128, HPC * DH], BF, name=f"wv{i}", tag=f"wv{i}") for i in range(DC)]

        nc.sync.dma_start(tri_sb[:], tri[:])
        nc.sync.dma_start(bo_sb[:], bo_r[:])
        nc.gpsimd.partition_broadcast(bo_bc[:], bo_sb[:])
        # unpack gathered weights: w_all rows[0:128]=d-chunks 0-3 of the
        # slice, rows[128:256]=chunks 4-7; col-blocks wq|wk|wv at 256/chunk,
        # wo at cols 3072:4096
        for i in range(DC):
            r = slice(0, 128) if i < 4 else slice(128, 256)
            cb = 256 * (i % 4)
            nc.sync.dma_start(wq_sb[i][:], w_all[r, cb:cb + 256])
            nc.sync.dma_start(wk_sb[i][:], w_all[r, 1024 + cb:1024 + cb + 256])
            nc.sync.dma_start(wv_sb[i][:], w_all[r, 2048 + cb:2048 + cb + 256])
        for p in range(PAIRS):
            nc.sync.dma_start(
                wo_sb[p][:], w_all[128 * p:128 * (p + 1), 3072:4096])
        for i in range(DC):
            nc.sync.dma_start(xts[i][:], xg[i * 128:(i + 1) * 128, :])

        def proj_qk_chunked(p, pool):
            qps = [pool.tile([128, QT], F32, name=f"qps{st}", tag=f"qk{st}") for st in range(NQT)]
            kps = [pool.tile([128, QT], F32, name=f"kps{st}", tag=f"qk{4 + st}") for st in range(NQT)]
            for i in range(DC):
                for st in range(NQT):
                    nc.tensor.matmul(
                        qps[st][:], wq_sb[i][:, p * 128:(p + 1) * 128],
                        xts[i][:, st * QT:(st + 1) * QT],
                        start=(i == 0), stop=(i == DC - 1))
                for st in range(NQT):
                    nc.tensor.matmul(
                        kps[st][:], wk_sb[i][:, p * 128:(p + 1) * 128],
                        xts[i][:, st * QT:(st + 1) * QT],
                        start=(i == 0), stop=(i == DC - 1))
            for st in range(NQT):
                nc.scalar.copy(qt_sb[p][:, st * QT:(st + 1) * QT], qps[st][:])
                nc.vector.tensor_copy(kt_sb[p][:, st * QT:(st + 1) * QT], kps[st][:])

        def proj_qk_chunk(p, st, pool):
            qp = pool.tile([128, QT], F32, name="qp", tag="qkseq")
            for i in range(DC):
                nc.tensor.matmul(
                    qp[:], wq_sb[i][:, p * 128:(p + 1) * 128],
                    xts[i][:, st * QT:(st + 1) * QT],
                    start=(i == 0), stop=(i == DC - 1))
            nc.vector.tensor_copy(qt_sb[p][:, st * QT:(st + 1) * QT], qp[:])
            kp = pool.tile([128, QT], F32, name="kp", tag="qkseq")
            for i in range(DC):
                nc.tensor.matmul(
                    kp[:], wk_sb[i][:, p * 128:(p + 1) * 128],
                    xts[i][:, st * QT:(st + 1) * QT],
                    start=(i == 0), stop=(i == DC - 1))
            nc.vector.tensor_copy(kt_sb[p][:, st * QT:(st + 1) * QT], kp[:])

        def attention(p, h, qt_i, scps, ctxps, att, attsm):
            hl = 2 * p + h
            r0, r1 = h * 64, h * 64 + 64
            q0 = qt_i * QT
            nkb = 4 * (qt_i + 1)
            # rows 0:DH ctx, row DH = l, rows DH+1.. scratch for transpose
            cps = ctxps.tile([DH + 33, QT], F32, name="cps", tag="cps")
            for g0 in range(0, nkb, 2):
                sp = scps.tile([128, 2 * QT], F32, name="sp", tag="sp")
                straddle2 = (g0 == 4 * qt_i + 2) and "masks" not in ABLATE
                if straddle2:
                    nc.tensor.matmul(
                        sp[:, 0:QT],
                        kt_sb[p][r0:r1, g0 * KB:(g0 + 1) * KB],
                        qt_sb[p][r0:r1, q0:q0 + QT],
                        start=True, stop=True)
                    nc.tensor.matmul(
                        sp[:, QT:QT + KB],
                        kt_sb[p][r0:r1, (g0 + 1) * KB:(g0 + 2) * KB],
                        qt_sb[p][r0:r1, q0 + 3 * KB:q0 + 4 * KB],
                        start=True, stop=True)
                else:
                    for u in range(2):
                        kb = g0 + u
                        nc.tensor.matmul(
                            sp[:, u * QT:(u + 1) * QT],
                            kt_sb[p][r0:r1, kb * KB:(kb + 1) * KB],
                            qt_sb[p][r0:r1, q0:q0 + QT],
                            start=True, stop=True)
                pt = att.tile([128, 2 * QT], BF, name="pt", tag="pt")
                if straddle2:
                    nc.scalar.activation(
                        pt[:, 2 * KB:QT + KB], sp[:, 2 * KB:QT + KB],
                        mybir.ActivationFunctionType.Exp, scale=float(SCALE))
                    nc.gpsimd.memset(pt[:, 0:2 * KB], 0.0)
                    for off in (2 * KB, QT):
                        nc.vector.tensor_mul(
                            pt[:, off:off + KB], pt[:, off:off + KB], tri_sb[:])
                else:
                    nc.scalar.activation(
                        pt[:], sp[:], mybir.ActivationFunctionType.Exp,
                        scale=float(SCALE))
                    if "masks" in ABLATE:
                        pass
                    elif g0 == 4 * qt_i:
                        nc.gpsimd.memset(pt[:, QT:QT + KB], 0.0)
                        for off in (0, QT + KB):
                            nc.vector.tensor_mul(
                                pt[:, off:off + KB], pt[:, off:off + KB], tri_sb[:])
                if straddle2:
                    nc.tensor.matmul(
                        cps[0:DH + 1, 3 * KB:QT],
                        vt_sb[g0 + 1][:, hl * (DH + 1):(hl + 1) * (DH + 1)],
                        pt[:, QT:QT + KB],
                        start=False, stop=False)
                    nc.tensor.matmul(
                        cps[0:DH + 1, :],
                        vt_sb[g0][:, hl * (DH + 1):(hl + 1) * (DH + 1)],
                        pt[:, 0:QT],
                        start=(g0 == 0), stop=(g0 + 1 == nkb - 1))
                else:
                    for u in range(2):
                        kb = g0 + u
                        nc.tensor.matmul(
                            cps[0:DH + 1, :],
                            vt_sb[kb][:, hl * (DH + 1):(hl + 1) * (DH + 1)],
                            pt[:, u * QT:(u + 1) * QT],
                            start=(kb == 0), stop=(kb == nkb - 1))
            # normalize via transposed reciprocal (free dim 16, not 512)
            tt = attsm.tile([32, QT], F32, name="tt", tag="tt")
            nc.vector.transpose(tt[:], cps[DH:DH + 32, :])
            rec = attsm.tile([32, QT], F32, name="rec", tag="rec")
            tv = tt.rearrange("p (b c) -> p b c", c=32)
            rv = rec.rearrange("p (b c) -> p b c", c=32)
            if "recip" in ABLATE:
                nc.vector.tensor_copy(rv[:, :, 0:1], tv[:, :, 0:1])
            else:
                nc.vector.reciprocal(rv[:, :, 0:1], tv[:, :, 0:1])
            rrow = attsm.tile([32, QT], F32, name="rrow", tag="rrow")
            nc.vector.transpose(rrow[:], rec[:])
            rb = attsm.tile([64, QT], F32, name="rb", tag="rb")
            nc.gpsimd.partition_broadcast(rb[:], rrow[0:1, :])
            nc.vector.tensor_mul(
                ctx_sb[p][r0:r1, q0:q0 + QT], cps[0:DH, :], rb[:])
            return cps

        def outproj(qt_i, ph3ps, ph3sb):
            """partial out-projection for one q tile -> rs_in[qt_i]; then
            quad ReduceScatter and DMA of this core's 128-row share."""
            for qb in range(qt_i * 4, qt_i * 4 + 4):
                os_ = ph3sb.tile([128, D], BF, name="os", tag="os")
                for nh in range(2):
                    op = ph3ps.tile([128, 512], F32, name="op", tag="op")
                    for p in range(PAIRS):
                        nc.tensor.matmul(
                            op[:], ctx_sb[p][:, qb * 128:(qb + 1) * 128],
                            wo_sb[p][:, nh * 512:(nh + 1) * 512],
                            start=(p == 0), stop=(p == PAIRS - 1))
                    if "outio" in ABLATE:
                        continue
                    nc.vector.tensor_add(
                        os_[:, nh * 512:(nh + 1) * 512], op[:],
                        bo_bc[:, nh * 512:(nh + 1) * 512])
                if "outdma" not in ABLATE and "outio" not in ABLATE:
                    r = (qb % 4) * 128
                    nc.sync.dma_start(rs_in[qt_i][r:r + 128, :], os_[:])
            if "outdma" not in ABLATE and "outio" not in ABLATE:
                nc.gpsimd.collective_compute(
                    "ReduceScatter", mybir.AluOpType.add, replica_groups=QUADS,
                    ins=[rs_in[qt_i].opt()], outs=[rs_out[qt_i].opt()])
                nc.gpsimd.dma_start(
                    out[qt_i * 128:(qt_i + 1) * 128, :], rs_out[qt_i][:])

        # phase A: q/k pair 0, chunk-pipelined against the unpack DMAs
        with tc.tile_pool(name="qk0ps", bufs=1, space="PSUM") as qk0ps:
            proj_qk_chunked(0, qk0ps)

        # phase B: V projection (own scope, closes before attention)
        skip_attn = "attn" in ABLATE
        with tc.tile_pool(name="vps", bufs=2, space="PSUM") as vps:
            for j in range(NKB):
                vp = vps.tile([128, HPC * DH], F32, name="vp", tag="vp")
                for i in range(DC):
                    nc.tensor.matmul(
                        vp[:], xts[i][:, j * 128:(j + 1) * 128], wv_sb[i][:],
                        start=(i == 0), stop=(i == DC - 1))
                vt_view = vt_sb[j].rearrange("p (h e) -> p h e", h=HPC)
                nc.vector.tensor_copy(
                    vt_view[:, :, 0:DH], vp.rearrange("p (h e) -> p h e", h=HPC))
                nc.gpsimd.memset(vt_view[:, :, DH:DH + 1], 1.0)

        # phase C: pair-0 attention with pair-1 q/k proj chunks interleaved
        with tc.tile_pool(name="att", bufs=6) as att, \
             tc.tile_pool(name="attsm", bufs=4) as attsm, \
             tc.tile_pool(name="scps", bufs=2, space="PSUM") as scps, \
             tc.tile_pool(name="ctxps", bufs=2, space="PSUM") as ctxps:

            with tc.tile_pool(name="qk1ps", bufs=2, space="PSUM") as qk1ps:
                for qt_i in range(NQT):
                    for h in range(2):
                        if not skip_attn:
                            attention(0, h, qt_i, scps, ctxps, att, attsm)
                    proj_qk_chunk(1, qt_i, qk1ps)

            # phase D: pair-1 attention, out-projection + RS interleaved
            with tc.tile_pool(name="ph3ps", bufs=2, space="PSUM") as ph3ps, \
                 tc.tile_pool(name="ph3sb", bufs=3) as ph3sb:
                for qt_i in range(NQT):
                    for h in range(2):
                        if not skip_attn:
                            attention(1, h, qt_i, scps, ctxps, att, attsm)
                    if "outproj" not in ABLATE and not skip_attn and qt_i > 0:
                        outproj(qt_i - 1, ph3ps, ph3sb)
                if "outproj" not in ABLATE and not skip_attn:
                    outproj(NQT - 1, ph3ps, ph3sb)

    nc.compile()
    return nc


_NC = None
PROFILE = False
TRACE_CORES = (0,)
LAST_RESULT = None


def _get_nc():
    global _NC
    if _NC is None:
        _NC = _build()
    return _NC


def kernel(x, Wq, Wk, Wv, Wo, bo):
    x = np.asarray(x, dtype=np.float32)
    Wq = np.asarray(Wq, dtype=np.float32)
    Wk = np.asarray(Wk, dtype=np.float32)
    Wv = np.asarray(Wv, dtype=np.float32)
    Wo = np.asarray(Wo, dtype=np.float32)
    bo = np.asarray(bo, dtype=np.float32)

    nc = _get_nc()

    in_maps = _prepare_in_maps(x, Wq, Wk, Wv, Wo, bo)

    global LAST_RESULT
    kw = {}
    if PROFILE:
        kw = dict(trace=True, trace_cores=list(TRACE_CORES))
    res = run_bass_kernel_spmd(nc, in_maps, core_ids=list(range(NCORES)), **kw)
    LAST_RESULT = res

    out = np.zeros((B, S, D), np.float32)
    for c in range(NCORES):
        b, g = divmod(c, 4)
        o = res.results[c]["out"].astype(np.float32)
        for j in range(NQT):
            r = 512 * j + 128 * g
            out[b, r:r + 128] = o[128 * j:128 * (j + 1)]
    return out


def _prepare_in_maps(x, Wq, Wk, Wv, Wo, bo):
    kk = np.arange(KB)[:, None]
    qq = np.arange(KB)[None, :]
    import ml_dtypes
    bf16 = ml_dtypes.bfloat16
    tri = (kk <= qq).astype(bf16)

    xT = [np.ascontiguousarray(x[b].T).astype(bf16) for b in range(B)]
    bo_row = np.ascontiguousarray(bo[None, :]).astype(np.float32)
    zeros_row = np.zeros((1, D), np.float32)

    in_maps = []
    for c in range(NCORES):
        b, g = divmod(c, 4)
        half = c // 4
        cs = slice(g * HPC * DH, (g + 1) * HPC * DH)
        wq_s, wk_s, wv_s = Wq[:, cs], Wk[:, cs], Wv[:, cs]
        wo_s = Wo[cs, :]
        rsl = slice(512 * half, 512 * (half + 1))
        # pack d-chunk-major: 4 chunks x [128,256] each of wq|wk|wv, then
        # wo rows [128*half:128*half+128]
        blocks = []
        for w in (wq_s, wk_s, wv_s):
            hrows = w[rsl]                      # [512, 256]
            for j in range(4):
                blocks.append(hrows[128 * j:128 * (j + 1)])
        blocks.append(Wo[cs, :][128 * half:128 * (half + 1)])
        w_pack = np.concatenate(blocks, axis=1).astype(bf16)  # [128, 4096]
        in_maps.append({
            "xs": np.ascontiguousarray(xT[b][XSH * g:XSH * (g + 1)]),
            "w_in": np.ascontiguousarray(w_pack),
            "bo_r": bo_row if g == 0 else zeros_row,
            "tri": tri,
        })
    return in_maps
